# revision 1
# baseline (speedup 1.0000x reference)
"""Trainium2 Bass kernel for nn_BaselineRvNNModel (collapsed RvNN/TreeLSTM).

Math (reference collapses to a per-node MLP + mean pool + classifier;
edge_index is dead):
    h1 = relu(x @ W1.T + b1)                      [N, H]
    g  = h1 @ W2.T + b2                           [N, H]   (pre-LN)
    gn = (g - mu) * rsqrt(var + eps)              per-row LN core
    iou = (gn * ln_w) @ W_iou.T + (ln_b @ W_iou.T + b_wiou + b_uiou)
    i, o, u = split(iou); c = sig(i)*tanh(u); hn = sig(o)*tanh(c)
    pooled = mean_rows(hn);  out = relu(pooled @ Wc1.T + bc1) @ Wc2.T + bc2

Distribution: data-parallel over nodes, 12500 rows/core on 8 cores,
AllReduce of the [H] pooled partial sum, replicated classifier.

Device layout: channels on partitions, rows on the free axis. x is
pre-transposed (and pre-tiled) host-side so no on-device transposes are
needed. LayerNorm's channel reduction is done with ones-vector matmuls on
the PE; rsqrt is computed as exp(-0.5*ln(v)) to stay within one extra ACT
table set; per-row stats are broadcast across partitions on GPSIMD.
"""

import numpy as np
import ml_dtypes

N_TOTAL = 100000
D = 768
H = 256
C = 4
NCORES = 8
LN_EPS = 1e-5

_CACHE = {}


def build_nc(npc, nt, ncores, use_f32_x=False, debug_taps=False, stage="full",
             ngroups=2):
    """Build the per-core Bass graph. npc = rows per core, nt = rows per tile."""
    from contextlib import ExitStack
    import concourse.bass as bass
    import concourse.bacc as bacc
    import concourse.tile as tile
    from concourse import mybir

    f32 = mybir.dt.float32
    f32r = mybir.dt.float32r
    bf16 = mybir.dt.bfloat16
    AF = mybir.ActivationFunctionType
    ALU = mybir.AluOpType

    ntiles = npc // nt
    assert ntiles * nt == npc
    KD = D // 128   # 6 contraction chunks for x
    KH = H // 128   # 2 chunks for H
    K3 = 3 * H // 128  # 6 output chunks for iou

    xdt = f32r if use_f32_x else bf16

    nc = bacc.Bacc("TRN2", target_bir_lowering=False, debug=False,
                   num_devices=ncores)

    # DRAM inputs. xtt is pre-tiled host-side: [ntiles, 128, KD, nt]
    xtt = nc.dram_tensor("xtt", [ntiles, 128, KD, nt], xdt, kind="ExternalInput")
    w1t = nc.dram_tensor("w1t", [D, H], xdt, kind="ExternalInput")        # W1.T
    b1d = nc.dram_tensor("b1d", [128, KH], f32, kind="ExternalInput")
    w2t = nc.dram_tensor("w2t", [H, H], bf16, kind="ExternalInput")       # W2.T
    b2d = nc.dram_tensor("b2d", [128, KH], f32, kind="ExternalInput")
    wiout = nc.dram_tensor("wiout", [H, 3 * H], bf16, kind="ExternalInput")  # (W_iou*ln_w).T
    c3d = nc.dram_tensor("c3d", [128, K3], f32, kind="ExternalInput")
    wc1t = nc.dram_tensor("wc1t", [H, H // 2], f32, kind="ExternalInput")  # Wc1.T/N
    bc1d = nc.dram_tensor("bc1d", [128, 1], f32, kind="ExternalInput")
    wc2t = nc.dram_tensor("wc2t", [H // 2, C], f32, kind="ExternalInput")  # Wc2.T
    bc2d = nc.dram_tensor("bc2d", [C, 1], f32, kind="ExternalInput")
    out_d = nc.dram_tensor("out", [C, 1], f32, kind="ExternalOutput")
    if debug_taps:
        dbg_g = nc.dram_tensor("dbg_g", [128, H // 128, npc], bf16,
                               kind="ExternalOutput")
        dbg_sst = nc.dram_tensor("dbg_sst", [npc // nt, 2, nt], bf16,
                                 kind="ExternalOutput")
        dbg_pool = nc.dram_tensor("dbg_pool", [128, H // 128, npc // nt], f32,
                                  kind="ExternalOutput")

    with tile.TileContext(nc) as tc, ExitStack() as ctx:
        # ---------------- constants (live whole kernel) ----------------
        pconst = ctx.enter_context(tc.tile_pool(name="consts", bufs=1))
        w1_sb = pconst.tile([128, KD, H], xdt)          # [128, k, m-chans]
        nc.sync.dma_start(w1_sb[:], w1t.ap().rearrange("(k p) m -> p k m", p=128))
        b1_sb = pconst.tile([128, KH], f32)
        nc.sync.dma_start(b1_sb[:], b1d.ap())
        w2_sb = pconst.tile([128, KH, H], bf16)
        nc.sync.dma_start(w2_sb[:], w2t.ap().rearrange("(k p) m -> p k m", p=128))
        b2_sb = pconst.tile([128, KH], f32)
        nc.sync.dma_start(b2_sb[:], b2d.ap())
        w3_sb = pconst.tile([128, KH, 3 * H], bf16)
        nc.gpsimd.dma_start(w3_sb[:],
                            wiout.ap().rearrange("(k p) m -> p k m", p=128))
        c3_sb = pconst.tile([128, K3], f32)
        nc.gpsimd.dma_start(c3_sb[:], c3d.ap())
        wc1_sb = pconst.tile([128, KH, H // 2], f32)
        nc.gpsimd.dma_start(wc1_sb[:],
                            wc1t.ap().rearrange("(k p) m -> p k m", p=128))
        bc1_sb = pconst.tile([128, 1], f32)
        nc.gpsimd.dma_start(bc1_sb[:], bc1d.ap())
        wc2_sb = pconst.tile([128, C], f32)
        nc.gpsimd.dma_start(wc2_sb[:], wc2t.ap())
        bc2_sb = pconst.tile([C, 1], f32)
        nc.gpsimd.dma_start(bc2_sb[:], bc2d.ap())
        ones_sb = pconst.tile([128, 1], bf16)
        nc.vector.memset(ones_sb[:], 1.0 / H)
        ones8_sb = pconst.tile([ncores, 1], f32)
        nc.vector.memset(ones8_sb[:], 1.0)
        eps_sb = pconst.tile([1, 1], f32)
        nc.vector.memset(eps_sb[:], LN_EPS)

        # persistent buffers
        pg = ctx.enter_context(tc.tile_pool(name="gbuf", bufs=1))
        gbuf = pg.tile([128, KH, npc], bf16)            # pre-LN activations
        accb = pg.tile([128, KH, nt], f32)              # pooled row accumulators
        nc.vector.memset(accb[:], 0.0)

        pdram = ctx.enter_context(tc.tile_pool(name="dram", bufs=1, space="DRAM"))
        statsd = pdram.tile([2, ntiles * nt], f32)
        ssd = pdram.tile([ntiles, 2, nt], bf16)         # s / s*mu rows
        ccin = pdram.tile([128, KH], f32)
        ccout = pdram.tile([ncores, 128, KH], f32)      # AllGather output

        # ======== grouped pipeline: A(g) -> stats(g) -> B(g), groups overlap ====
        if ngroups == 2:
            # uneven split: leftover B(g0) tiles keep the PE busy while
            # stats(g1) is computed
            c0 = min(ntiles - 1, (ntiles * 3) // 5)
            groups = [list(range(c0)), list(range(c0, ntiles))]
        else:
            gsz = (ntiles + ngroups - 1) // ngroups
            groups = [list(range(g * gsz, min((g + 1) * gsz, ntiles)))
                      for g in range(ngroups)]
            groups = [g for g in groups if g]

        with tc.tile_pool(name="xin", bufs=3) as px, \
             tc.tile_pool(name="h1", bufs=4) as ph1, \
             tc.tile_pool(name="gsq", bufs=4) as pgs, \
             tc.tile_pool(name="stage", bufs=4) as pstg, \
             tc.tile_pool(name="stats", bufs=2) as pst, \
             tc.tile_pool(name="gn", bufs=4) as pgn, \
             tc.tile_pool(name="gt", bufs=6) as pgt, \
             tc.tile_pool(name="hnscr", bufs=2) as phs, \
             tc.tile_pool(name="psA1", bufs=2, space="PSUM") as pps1, \
             tc.tile_pool(name="psA2", bufs=2, space="PSUM") as pps2, \
             tc.tile_pool(name="psB", bufs=4, space="PSUM") as ppsb:

            def phase_a(j):
                if j < 2:
                    # split first tiles per k-chunk so mm1 starts after 128KB
                    xks = [px.tile([128, nt], xdt, tag=f"x0k{k}", bufs=2,
                                   name=f"xs{j}k{k}") for k in range(KD)]
                    for k in range(KD):
                        nc.sync.dma_start(xks[k][:], xtt.ap()[j, :, k, :])
                    xsl = [xks[k][:] for k in range(KD)]
                else:
                    xs = px.tile([128, KD, nt], xdt, tag="x", name=f"xs{j}")
                    nc.sync.dma_start(xs[:], xtt.ap()[j])
                    xsl = [xs[:, k, :] for k in range(KD)]
                h1s = []
                for m in range(KH):
                    pm = pps1.tile([128, nt], f32, tag="h1p", name=f"ph1_{j}_{m}")
                    for k in range(KD):
                        nc.tensor.matmul(
                            pm[:], w1_sb[:, k, m * 128:(m + 1) * 128],
                            xsl[k], start=(k == 0), stop=(k == KD - 1))
                    h1 = ph1.tile([128, nt], bf16, tag="h1", name=f"h1_{j}_{m}")
                    nc.scalar.activation(h1[:], pm[:], AF.Relu,
                                         bias=b1_sb[:, m:m + 1])
                    h1s.append(h1)
                for m in range(KH):
                    pm = pps2.tile([128, nt], f32, tag="h2p", name=f"ph2_{j}_{m}")
                    for k in range(KH):
                        nc.tensor.matmul(
                            pm[:], w2_sb[:, k, m * 128:(m + 1) * 128],
                            h1s[k][:], start=(k == 0), stop=(k == KH - 1))
                    gv = gbuf[:, m, j * nt:(j + 1) * nt]
                    nc.vector.tensor_scalar(
                        out=gv, in0=pm[:], scalar1=b2_sb[:, m:m + 1],
                        scalar2=None, op0=ALU.add)
                gsq = pgs.tile([128, KH, nt], bf16, tag="gsq", name=f"gsq{j}")
                for m in range(KH):
                    nc.vector.tensor_tensor(
                        out=gsq[:, m, :], in0=gbuf[:, m, j * nt:(j + 1) * nt],
                        in1=gbuf[:, m, j * nt:(j + 1) * nt], op=ALU.mult)
                pmu = pps2.tile([1, nt], f32, tag="h2p", name=f"pmu{j}")
                for m in range(KH):
                    nc.tensor.matmul(pmu[:], ones_sb[:],
                                     gbuf[:, m, j * nt:(j + 1) * nt],
                                     start=(m == 0), stop=(m == KH - 1))
                pmsq = pps2.tile([1, nt], f32, tag="h2p", name=f"pmsq{j}")
                for m in range(KH):
                    nc.tensor.matmul(pmsq[:], ones_sb[:], gsq[:, m, :],
                                     start=(m == 0), stop=(m == KH - 1))
                stg = pstg.tile([1, 2, nt], f32, tag="stg", name=f"stg{j}")
                nc.scalar.activation(stg[:, 0, :], pmu[:], AF.Copy)
                # msq + eps (so var' = msq' - mu^2 = var + eps)
                nc.scalar.activation(stg[:, 1, :], pmsq[:], AF.Identity,
                                     bias=eps_sb[:])
                nc.gpsimd.dma_start(statsd[:, j * nt:(j + 1) * nt], stg[:])

            def phase_stats(g, tl):
                # s = exp(-0.5*ln(var+eps)) for this group's rows
                ng = len(tl)
                j0 = tl[0]
                w = slice(j0 * nt, (tl[-1] + 1) * nt)
                mu2 = pst.tile([ng, nt], f32, tag="mu2", name=f"mu2g{g}")
                nc.gpsimd.dma_start(
                    mu2[:], statsd[0:1, w].rearrange("o (j t) -> (o j) t", j=ng))
                msq2 = pst.tile([ng, nt], f32, tag="msq2", name=f"msq2g{g}")
                nc.gpsimd.dma_start(
                    msq2[:], statsd[1:2, w].rearrange("o (j t) -> (o j) t", j=ng))
                musq = pst.tile([ng, nt], f32, tag="musq", name=f"musqg{g}")
                nc.scalar.activation(musq[:], mu2[:], AF.Square)
                varr = pst.tile([ng, nt], f32, tag="varr", name=f"varrg{g}")
                nc.vector.tensor_tensor(out=varr[:], in0=msq2[:], in1=musq[:],
                                        op=ALU.subtract)
                lnv = pst.tile([ng, nt], f32, tag="lnv", name=f"lnvg{g}")
                nc.scalar.activation(lnv[:], varr[:], AF.Ln)
                sst = pst.tile([ng, 2, nt], bf16, tag="sst", name=f"sstg{g}")
                nc.scalar.activation(sst[:, 0, :], lnv[:], AF.Exp, scale=-0.5)
                nc.vector.tensor_tensor(out=sst[:, 1, :], in0=sst[:, 0, :],
                                        in1=mu2[:], op=ALU.mult)
                nc.gpsimd.dma_start(ssd[j0:j0 + ng], sst[:])

            def phase_b(j):
                jw = slice(j * nt, (j + 1) * nt)
                sb = pgn.tile([128, 2, nt], bf16, tag="sb", name=f"sb{j}")
                nc.gpsimd.dma_start(
                    sb[:], ssd[j:j + 1, :, :].partition_broadcast(128))
                gn = pgn.tile([128, KH, nt], bf16, tag="gn", name=f"gn{j}")
                for m in range(KH):
                    tt = pgt.tile([128, nt], bf16, tag="tmp", name=f"tt{j}_{m}")
                    nc.vector.tensor_tensor(out=tt[:], in0=gbuf[:, m, jw],
                                            in1=sb[:, 0, :], op=ALU.mult)
                    nc.vector.tensor_tensor(out=gn[:, m, :], in0=tt[:],
                                            in1=sb[:, 1, :], op=ALU.subtract)
                for m in range(KH):
                    pious = []
                    for m3 in (m, 2 + m, 4 + m):
                        pio = ppsb.tile([128, nt], f32, tag="iou",
                                        name=f"pio{j}_{m3}")
                        for k in range(KH):
                            nc.tensor.matmul(
                                pio[:], w3_sb[:, k, m3 * 128:(m3 + 1) * 128],
                                gn[:, k, :], start=(k == 0), stop=(k == KH - 1))
                        pious.append(pio)
                    pi, po, pu = pious
                    si = pgt.tile([128, nt], bf16, tag="si", name=f"si{j}_{m}")
                    nc.scalar.activation(si[:], pi[:], AF.Sigmoid,
                                         bias=c3_sb[:, m:m + 1])
                    tu = pgt.tile([128, nt], bf16, tag="tu", name=f"tu{j}_{m}")
                    nc.scalar.activation(tu[:], pu[:], AF.Tanh,
                                         bias=c3_sb[:, 4 + m:5 + m])
                    so = pgt.tile([128, nt], bf16, tag="so", name=f"so{j}_{m}")
                    nc.scalar.activation(so[:], po[:], AF.Sigmoid,
                                         bias=c3_sb[:, 2 + m:3 + m])
                    cpre = pgt.tile([128, nt], bf16, tag="cpre",
                                    name=f"cp{j}_{m}")
                    nc.vector.tensor_tensor(out=cpre[:], in0=si[:], in1=tu[:],
                                            op=ALU.mult)
                    tc_t = pgt.tile([128, nt], bf16, tag="tc", name=f"tct{j}_{m}")
                    nc.scalar.activation(tc_t[:], cpre[:], AF.Tanh)
                    hns = phs.tile([128, nt], bf16, tag="hns", name=f"hn{j}_{m}")
                    nc.vector.tensor_tensor(out=hns[:], in0=so[:], in1=tc_t[:],
                                            op=ALU.mult)
                    nc.vector.tensor_tensor(out=accb[:, m, :],
                                            in0=accb[:, m, :], in1=hns[:],
                                            op=ALU.add)

            if stage == "A":
                for j in range(ntiles):
                    phase_a(j)
            else:
                # software pipeline: A(g0); stats(g0); then interleave
                # B(g) tiles with A(g+1) tiles; stats(g+1) after A(g+1).
                for j in groups[0]:
                    phase_a(j)
                phase_stats(0, groups[0])
                for g in range(1, len(groups)):
                    prev, cur = groups[g - 1], groups[g]
                    # pair A(cur) with B(prev); emit stats(cur) right after
                    # the last A so leftover B(prev) tiles hide its latency
                    for i in range(len(cur)):
                        phase_a(cur[i])
                        if i < len(prev):
                            phase_b(prev[i])
                    phase_stats(g, cur)
                    for i in range(len(cur), len(prev)):
                        phase_b(prev[i])
                for j in groups[-1]:
                    phase_b(j)

        if stage == "A":
            nc.sync.dma_start(out_d.ap(), statsd[0:1, 0:C])

        # ================= pool + all-reduce + classifier =================
        if debug_taps:
            nc.sync.dma_start(dbg_g.ap(), gbuf[:])
            nc.sync.dma_start(dbg_sst.ap(), ssd[:])
            nc.sync.dma_start(dbg_pool.ap(), accb[:, :, 0:ntiles])

        if stage == "B":
            nc.sync.dma_start(out_d.ap(), accb[0:C, 0, 0:1])

        if stage in ("full", "noar"):
          with tc.tile_pool(name="fin", bufs=1) as pf, \
             tc.tile_pool(name="psF", bufs=2, space="PSUM") as ppsf:
            pv = pf.tile([128, KH], f32)
            for m in range(KH):
                nc.vector.tensor_reduce(out=pv[:, m:m + 1], in_=accb[:, m, :],
                                        axis=mybir.AxisListType.X,
                                        op=ALU.add)
            # pv laid out DRAM-contiguously: ccin flat = [p0k0,p0k1,p1k0,...]
            nc.sync.dma_start(ccin[:], pv[:])
            if stage == "noar":
                for r in range(ncores):
                    nc.sync.dma_start(ccout[r:r + 1], ccin[:])
            else:
                nc.gpsimd.collective_compute(
                    "AllGather", ALU.bypass,
                    replica_groups=[list(range(ncores))],
                    ins=[ccin[:].opt()], outs=[ccout[:].opt()])
            # one clean DMA: rank r -> partition r, free dim = p*KH+k
            g8 = pf.tile([ncores, 128 * KH], f32)
            nc.sync.dma_start(
                g8[:], ccout[:].rearrange("r p k -> r (p k)"))
            # pooled[p, k] = sum_r g8[r, p*KH+k] via ones-matmul on PE
            pps = ppsf.tile([128, KH], f32)
            g8v = g8[:].rearrange("r (p k) -> r p k", p=128)
            for k in range(KH):
                nc.tensor.matmul(pps[:, k:k + 1], g8v[:, :, k], ones8_sb[:],
                                 start=True, stop=True)
            ps = pf.tile([128, KH], f32)
            nc.vector.tensor_copy(ps[:], pps[:])
            pz = ppsf.tile([128, 1], f32)
            for k in range(KH):
                nc.tensor.matmul(pz[:], wc1_sb[:, k, :], ps[:, k:k + 1],
                                 start=(k == 0), stop=(k == KH - 1))
            zz = pf.tile([128, 1], f32)
            nc.vector.tensor_scalar(out=zz[:], in0=pz[:], scalar1=bc1_sb[:],
                                    scalar2=0.0, op0=ALU.add, op1=ALU.max)
            po2 = ppsf.tile([C, 1], f32)
            nc.tensor.matmul(po2[:], wc2_sb[:], zz[:], start=True, stop=True)
            oo = pf.tile([C, 1], f32)
            nc.vector.tensor_scalar(out=oo[:], in0=po2[:], scalar1=bc2_sb[:],
                                    scalar2=None, op0=ALU.add)
            nc.sync.dma_start(out_d.ap(), oo[:])

    nc.compile()
    return nc


def host_prep(inputs, npc, nt, ncores, use_f32_x=False):
    """Shard + lay out inputs for the device. Returns in_maps (list per core)."""
    bf16 = ml_dtypes.bfloat16
    xdt = np.float32 if use_f32_x else bf16
    ntiles = npc // nt
    KH = H // 128
    K3 = 3 * H // 128

    x = np.asarray(inputs["x"], np.float32)
    W1 = np.asarray(inputs["W1"], np.float32)
    b1 = np.asarray(inputs["b1"], np.float32)
    W2 = np.asarray(inputs["W2"], np.float32)
    b2 = np.asarray(inputs["b2"], np.float32)
    ln_w = np.asarray(inputs["ln_w"], np.float32)
    ln_b = np.asarray(inputs["ln_b"], np.float32)
    W_iou = np.asarray(inputs["W_iou"], np.float32)
    b_wiou = np.asarray(inputs["b_wiou"], np.float32)
    b_uiou = np.asarray(inputs["b_uiou"], np.float32)
    Wc1 = np.asarray(inputs["Wc1"], np.float32)
    bc1 = np.asarray(inputs["bc1"], np.float32)
    Wc2 = np.asarray(inputs["Wc2"], np.float32)
    bc2 = np.asarray(inputs["bc2"], np.float32)

    shared = {
        "w1t": np.ascontiguousarray(W1.T).astype(xdt),
        "b1d": np.ascontiguousarray(b1.reshape(KH, 128).T),
        "w2t": np.ascontiguousarray(W2.T).astype(bf16),
        "b2d": np.ascontiguousarray(b2.reshape(KH, 128).T),
        "wiout": np.ascontiguousarray((W_iou * ln_w[None, :]).T).astype(bf16),
        "c3d": np.ascontiguousarray(
            (W_iou @ ln_b + b_wiou + b_uiou).astype(np.float32)
            .reshape(K3, 128).T),
        "wc1t": np.ascontiguousarray(Wc1.T).astype(np.float32) / float(x.shape[0]),
        "bc1d": np.ascontiguousarray(bc1.reshape(128, 1)),
        "wc2t": np.ascontiguousarray(Wc2.T).astype(np.float32),
        "bc2d": np.ascontiguousarray(bc2.reshape(C, 1)),
    }
    in_maps = []
    for c in range(ncores):
        xs = x[c * npc:(c + 1) * npc]                      # [npc, D]
        # [ntiles, 128, KD, nt]: tile j, partition p, d-chunk k, row t
        xtt = (xs.reshape(ntiles, nt, D // 128, 128)
               .transpose(0, 3, 2, 1).astype(xdt))
        in_maps.append({"xtt": np.ascontiguousarray(xtt), **shared})
    return in_maps


def kernel(**inputs):
    from concourse.bass_utils import run_bass_kernel_spmd

    npc = N_TOTAL // NCORES
    nt = 500
    key = (npc, nt, NCORES)
    if key not in _CACHE:
        _CACHE[key] = build_nc(npc, nt, NCORES)
    nc = _CACHE[key]
    in_maps = host_prep(inputs, npc, nt, NCORES)
    res = run_bass_kernel_spmd(nc, in_maps, core_ids=list(range(NCORES)))
    return np.ascontiguousarray(
        res.results[0]["out"].reshape(1, C).astype(np.float32))



# revision 9
# speedup vs baseline: 1.1406x; 1.1406x over previous
"""Trainium2 Bass kernel for nn_BaselineRvNNModel (collapsed RvNN/TreeLSTM).

Math (reference collapses to a per-node MLP + mean pool + classifier;
edge_index is dead):
    h1 = relu(x @ W1.T + b1)                      [N, H]
    g  = h1 @ W2.T + b2                           [N, H]   (pre-LN)
    gn = (g - mu) * rsqrt(var + eps)              per-row LN core
    iou = gn @ (W_iou*ln_w).T + (W_iou@ln_b + b_wiou + b_uiou)
    i, o, u = split(iou); c = sig(i)*tanh(u); hn = sig(o)*tanh(c)
    pooled = mean_rows(hn);  out = relu(pooled @ Wc1.T + bc1) @ Wc2.T + bc2

Distribution: data-parallel over nodes, 12500 rows/core on 8 cores. Each
core emits its partial pooled sum [128, 2] f32; the host sums the 8
partials and applies the tiny classifier (256->128->4) in fp32.

Precision: mm1 (x@W1.T) and mm3 (gn@Wio.T) run as fp8-e4m3 DoubleRow
matmuls (2 contraction rows/cycle); mm2 and the LN-stats matmuls stay
bf16 (mm2 in fp8 dominates the error budget). Measured host-sim rel err
of this mix: ~4.7e-3 (tolerance 2e-2).

Scales (powers of 2): x*16, W1*16 -> pm1 = 256*xW1; h1 stored as
256*relu bf16; g = pm2/256 + b2 true-scale bf16; s8 = 8*rsqrt(var+eps);
gns = fp8(g*s8 - 8*s*mu) = 8*gn; W3*16 -> pm3 = 128*iou; gates use ACT
scale=1/128 with per-chunk bias.
"""

import numpy as np
import ml_dtypes

N_TOTAL = 100000
D = 768
H = 256
C = 4
NCORES = 8
LN_EPS = 1e-5

_CACHE = {}


def build_nc(npc, nt, ncores, ngroups=2):
    from contextlib import ExitStack
    import concourse.bass as bass
    import concourse.bacc as bacc
    import concourse.tile as tile
    from concourse import mybir

    f32 = mybir.dt.float32
    bf16 = mybir.dt.bfloat16
    f8 = mybir.dt.float8e4
    AF = mybir.ActivationFunctionType
    ALU = mybir.AluOpType
    DR = mybir.MatmulPerfMode.DoubleRow

    ntiles = npc // nt
    assert ntiles * nt == npc
    KD = D // 128            # 6 contraction chunks for x
    KH = H // 128            # 2 chunks for H
    NPAIR = KD // 2          # 3 DoubleRow pairs for mm1

    nc = bacc.Bacc("TRN2", target_bir_lowering=False, debug=False,
                   num_devices=ncores)

    # DRAM inputs (all pre-tiled host-side to contiguous device layouts).
    xtt = nc.dram_tensor("xtt", [ntiles, 128, KD, nt], f8, kind="ExternalInput")
    w1d = nc.dram_tensor("w1d", [128, KD, H], f8, kind="ExternalInput")
    b1d = nc.dram_tensor("b1d", [128, KH], f32, kind="ExternalInput")   # b1*256
    w2d = nc.dram_tensor("w2d", [128, KH, H], bf16, kind="ExternalInput")
    b2d = nc.dram_tensor("b2d", [128, KH], f32, kind="ExternalInput")
    w3d = nc.dram_tensor("w3d", [128, KH, 3 * H], f8, kind="ExternalInput")
    c3d = nc.dram_tensor("c3d", [128, 3 * KH], f32, kind="ExternalInput")
    out_d = nc.dram_tensor("out", [128, KH], f32, kind="ExternalOutput")

    with tile.TileContext(nc) as tc, ExitStack() as ctx:
        # ---------------- constants (live whole kernel) ----------------
        pconst = ctx.enter_context(tc.tile_pool(name="consts", bufs=1))
        w1_sb = pconst.tile([128, KD, H], f8)
        nc.sync.dma_start(w1_sb[:], w1d.ap())
        b1_sb = pconst.tile([128, KH], f32)
        nc.sync.dma_start(b1_sb[:], b1d.ap())
        w2_sb = pconst.tile([128, KH, H], bf16)
        nc.sync.dma_start(w2_sb[:], w2d.ap())
        b2_sb = pconst.tile([128, KH], f32)
        nc.sync.dma_start(b2_sb[:], b2d.ap())
        w3_sb = pconst.tile([128, KH, 3 * H], f8)
        nc.gpsimd.dma_start(w3_sb[:], w3d.ap())
        c3_sb = pconst.tile([128, 3 * KH], f32)
        nc.gpsimd.dma_start(c3_sb[:], c3d.ap())
        ones_sb = pconst.tile([128, 32], bf16)
        nc.vector.memset(ones_sb[:], 1.0 / H)
        eps_sb = pconst.tile([16, 1], f32)
        nc.vector.memset(eps_sb[:], LN_EPS)
        ln8_sb = pconst.tile([16, 1], f32)
        nc.vector.memset(ln8_sb[:], float(np.log(8.0)))

        # persistent buffers
        pg = ctx.enter_context(tc.tile_pool(name="gbuf", bufs=1))
        gbuf = pg.tile([128, KH, npc], bf16)
        accslab = pg.tile([128, KH, 32], f32)
        nc.vector.memset(accslab[:], 0.0)

        pdram = ctx.enter_context(tc.tile_pool(name="dram", bufs=1, space="DRAM"))
        statsd = pdram.tile([ntiles, 2, nt], bf16)   # per-tile {mu, msq}
        ssd = pdram.tile([ntiles, 2, nt], bf16)      # per-tile {8s, 8*s*mu}

        # groups for the A -> stats -> B software pipeline
        if ngroups == 2:
            c0 = min(ntiles - 1, (ntiles * 3) // 5)
            groups = [list(range(c0)), list(range(c0, ntiles))]
        else:
            gsz = (ntiles + ngroups - 1) // ngroups
            groups = [list(range(g * gsz, min((g + 1) * gsz, ntiles)))
                      for g in range(ngroups)]
            groups = [g for g in groups if g]

        with tc.tile_pool(name="xin", bufs=3) as px, \
             tc.tile_pool(name="h1", bufs=2) as ph1, \
             tc.tile_pool(name="gsq", bufs=2) as pgs, \
             tc.tile_pool(name="stg", bufs=3) as pstg, \
             tc.tile_pool(name="stats", bufs=2) as pst, \
             tc.tile_pool(name="gn", bufs=3) as pgn, \
             tc.tile_pool(name="gt", bufs=2) as pgt, \
             tc.tile_pool(name="hnscr", bufs=2) as phs, \
             tc.tile_pool(name="psA", bufs=3, space="PSUM") as ppsa, \
             tc.tile_pool(name="psU", bufs=1, space="PSUM") as ppsu, \
             tc.tile_pool(name="psIO", bufs=4, space="PSUM") as ppsio:

            def phase_a(j):
                jw = slice(j * nt, (j + 1) * nt)
                if j < 2:
                    xks = [px.tile([128, 2, nt], f8, tag=f"x0k{k}", bufs=2,
                                   name=f"xs{j}k{k}") for k in range(NPAIR)]
                    for k in range(NPAIR):
                        nc.sync.dma_start(xks[k][:],
                                          xtt.ap()[j, :, 2 * k:2 * k + 2, :])
                    xsl = [xks[k][:] for k in range(NPAIR)]
                else:
                    xs = px.tile([128, KD, nt], f8, tag="x", name=f"xs{j}")
                    nc.sync.dma_start(xs[:], xtt.ap()[j])
                    xsl = [xs[:, 2 * k:2 * k + 2, :] for k in range(NPAIR)]
                # mm1 (fp8 DoubleRow) + h1 relu store
                h1 = ph1.tile([128, KH, nt], bf16, tag="h1", name=f"h1_{j}")
                pms = []
                for m in range(KH):
                    pm = ppsa.tile([128, 1, nt], f32, tag="A", name=f"pa1_{j}_{m}")
                    for k in range(NPAIR):
                        nc.tensor.matmul(
                            pm[:, 0, :],
                            w1_sb[:, 2 * k:2 * k + 2, m * 128:(m + 1) * 128],
                            xsl[k], start=(k == 0), stop=(k == NPAIR - 1),
                            perf_mode=DR)
                    pms.append(pm)
                for m in range(KH):
                    nc.vector.tensor_scalar(
                        out=h1[:, m, :], in0=pms[m][:, 0, :],
                        scalar1=b1_sb[:, m:m + 1], scalar2=0.0,
                        op0=ALU.add, op1=ALU.max)
                # mm2 (bf16) + g store
                pm2s = []
                for m in range(KH):
                    pm = ppsa.tile([128, 1, nt], f32, tag="A", name=f"pa2_{j}_{m}")
                    for k in range(KH):
                        nc.tensor.matmul(
                            pm[:, 0, :], w2_sb[:, k, m * 128:(m + 1) * 128],
                            h1[:, k, :], start=(k == 0), stop=(k == KH - 1))
                    pm2s.append(pm)
                for m in range(KH):
                    nc.vector.tensor_scalar(
                        out=gbuf[:, m, jw], in0=pm2s[m][:, 0, :],
                        scalar1=1.0 / 256.0, scalar2=b2_sb[:, m:m + 1],
                        op0=ALU.mult, op1=ALU.add)
                # gsq + stats matmuls (bf16) -> {mu, msq} psum -> sbuf -> DRAM
                gsq = pgs.tile([128, KH, nt], bf16, tag="gsq", name=f"gsq{j}")
                nc.vector.tensor_tensor(out=gsq[:], in0=gbuf[:, :, jw],
                                        in1=gbuf[:, :, jw], op=ALU.mult)
                psmu = ppsa.tile([128, 1, nt], f32, tag="A", name=f"psmu_{j}")
                for m in range(KH):
                    nc.tensor.matmul(psmu[0:32, 0, :], ones_sb[:],
                                     gbuf[:, m, jw],
                                     start=(m == 0), stop=(m == KH - 1))
                pssq = ppsa.tile([128, 1, nt], f32, tag="A", name=f"pssq_{j}")
                for m in range(KH):
                    nc.tensor.matmul(pssq[0:32, 0, :], ones_sb[:], gsq[:, m, :],
                                     start=(m == 0), stop=(m == KH - 1))
                stg = pstg.tile([64, nt], bf16, tag="stg", name=f"stg{j}")
                nc.vector.tensor_copy(stg[0:32, :], psmu[0:32, 0, :])
                nc.vector.tensor_copy(stg[32:64, :], pssq[0:32, 0, :])
                nc.gpsimd.dma_start(statsd[j], stg[31:33, :])

            def phase_stats(g, tl):
                ng = len(tl)
                j0 = tl[0]
                mu2 = pst.tile([ng, nt], bf16, tag="mu2", name=f"mu2g{g}")
                nc.gpsimd.dma_start(
                    mu2[:], statsd[j0:j0 + ng, 0:1, :]
                    .rearrange("j o t -> (j o) t"))
                ms2 = pst.tile([ng, nt], bf16, tag="ms2", name=f"ms2g{g}")
                nc.gpsimd.dma_start(
                    ms2[:], statsd[j0:j0 + ng, 1:2, :]
                    .rearrange("j o t -> (j o) t"))
                musq = pst.tile([ng, nt], f32, tag="musq", name=f"musqg{g}")
                nc.scalar.activation(musq[:], mu2[:], AF.Square)
                varr = pst.tile([ng, nt], f32, tag="varr", name=f"varrg{g}")
                nc.vector.tensor_tensor(out=varr[:], in0=ms2[:], in1=musq[:],
                                        op=ALU.subtract)
                lnv = pst.tile([ng, nt], f32, tag="lnv", name=f"lnvg{g}")
                nc.scalar.activation(lnv[:], varr[:], AF.Ln,
                                     bias=eps_sb[0:ng, :])
                sst = pst.tile([ng, 2, nt], bf16, tag="sst", name=f"sstg{g}")
                nc.scalar.activation(sst[:, 0, :], lnv[:], AF.Exp, scale=-0.5,
                                     bias=ln8_sb[0:ng, :])
                nc.vector.tensor_tensor(out=sst[:, 1, :], in0=sst[:, 0, :],
                                        in1=mu2[:], op=ALU.mult)
                nc.gpsimd.dma_start(ssd[j0:j0 + ng], sst[:])

            def phase_b(j):
                jw = slice(j * nt, (j + 1) * nt)
                sb = pgn.tile([128, 2, nt], bf16, tag="sb", name=f"sb{j}")
                nc.gpsimd.dma_start(
                    sb[:], ssd[j:j + 1, :, :].partition_broadcast(128))
                gs = pgn.tile([128, KH, nt], bf16, tag="gs", name=f"gs{j}")
                for m in range(KH):
                    nc.vector.tensor_tensor(out=gs[:, m, :],
                                            in0=gbuf[:, m, jw],
                                            in1=sb[:, 0, :], op=ALU.mult)
                gns = pgn.tile([128, KH, nt], f8, tag="gns", name=f"gns{j}")
                for m in range(KH):
                    nc.vector.tensor_tensor(out=gns[:, m, :], in0=gs[:, m, :],
                                            in1=sb[:, 1, :], op=ALU.subtract)
                # mm3 (fp8 DoubleRow): output chunk order [i0,o0 | i1,o1 | u0,u1]
                tu = pgt.tile([128, KH, nt], bf16, tag="tu", name=f"tu{j}")
                sis = []
                for m in range(KH):
                    pu = ppsu.tile([128, 1, nt], f32, tag="U", name=f"pu{j}_{m}")
                    nc.tensor.matmul(
                        pu[:, 0, :],
                        w3_sb[:, :, (4 + m) * 128:(5 + m) * 128],
                        gns[:], start=True, stop=True, perf_mode=DR)
                    nc.scalar.activation(tu[:, m, :], pu[:, 0, :], AF.Tanh,
                                         bias=c3_sb[:, 4 + m:5 + m],
                                         scale=1.0 / 128.0)
                    pii = ppsio.tile([128, 1, nt], f32, tag="IO",
                                     name=f"pii{j}_{m}")
                    nc.tensor.matmul(
                        pii[:, 0, :],
                        w3_sb[:, :, (2 * m) * 128:(2 * m + 1) * 128],
                        gns[:], start=True, stop=True, perf_mode=DR)
                    pio = ppsio.tile([128, 1, nt], f32, tag="IO",
                                     name=f"pio{j}_{m}")
                    nc.tensor.matmul(
                        pio[:, 0, :],
                        w3_sb[:, :, (2 * m + 1) * 128:(2 * m + 2) * 128],
                        gns[:], start=True, stop=True, perf_mode=DR)
                    siso = pgt.tile([128, 2, nt], bf16, tag="siso",
                                    name=f"siso{j}_{m}")
                    nc.scalar.activation(siso[:, 0, :], pii[:, 0, :],
                                         AF.Sigmoid,
                                         bias=c3_sb[:, 2 * m:2 * m + 1],
                                         scale=1.0 / 128.0)
                    nc.scalar.activation(siso[:, 1, :], pio[:, 0, :],
                                         AF.Sigmoid,
                                         bias=c3_sb[:, 2 * m + 1:2 * m + 2],
                                         scale=1.0 / 128.0)
                    sis.append(siso)
                cp = pgt.tile([128, KH, nt], bf16, tag="cp", name=f"cp{j}")
                for m in range(KH):
                    nc.gpsimd.tensor_tensor(out=cp[:, m, :],
                                            in0=sis[m][:, 0, :],
                                            in1=tu[:, m, :], op=ALU.mult)
                tc_t = pgt.tile([128, KH, nt], bf16, tag="tc", name=f"tc{j}")
                nc.scalar.activation(tc_t[:], cp[:], AF.Tanh)
                for m in range(KH):
                    hs = phs.tile([128, nt], bf16, tag="hs", name=f"hs{j}_{m}")
                    nc.vector.scalar_tensor_tensor(
                        out=hs[:], in0=sis[m][:, 1, :], scalar=1.0,
                        in1=tc_t[:, m, :], op0=ALU.mult, op1=ALU.mult,
                        accum_out=accslab[:, m, j:j + 1])

            # software pipeline: A(g0); stats(g0); interleave B(g) with A(g+1)
            for j in groups[0]:
                phase_a(j)
            phase_stats(0, groups[0])
            for g in range(1, len(groups)):
                prev, cur = groups[g - 1], groups[g]
                for i in range(len(cur)):
                    phase_a(cur[i])
                    if i < len(prev):
                        phase_b(prev[i])
                phase_stats(g, cur)
                for i in range(len(cur), len(prev)):
                    phase_b(prev[i])
            for j in groups[-1]:
                phase_b(j)

        # ---------------- partial pooled sum -> DRAM ----------------
        with tc.tile_pool(name="fin", bufs=1) as pf:
            pv = pf.tile([128, KH, 1], f32)
            nc.vector.tensor_reduce(out=pv[:], in_=accslab[:],
                                    axis=mybir.AxisListType.X, op=ALU.add)
            nc.sync.dma_start(out_d.ap(), pv[:, :, 0])

    nc.compile()
    return nc


def host_prep(inputs, npc, nt, ncores):
    """Shard + lay out inputs for the device. Returns (in_maps, host_ctx)."""
    bf16 = ml_dtypes.bfloat16
    f8 = ml_dtypes.float8_e4m3
    ntiles = npc // nt
    KH = H // 128

    x = np.asarray(inputs["x"], np.float32)
    W1 = np.asarray(inputs["W1"], np.float32)
    b1 = np.asarray(inputs["b1"], np.float32)
    W2 = np.asarray(inputs["W2"], np.float32)
    b2 = np.asarray(inputs["b2"], np.float32)
    ln_w = np.asarray(inputs["ln_w"], np.float32)
    ln_b = np.asarray(inputs["ln_b"], np.float32)
    W_iou = np.asarray(inputs["W_iou"], np.float32)
    b_wiou = np.asarray(inputs["b_wiou"], np.float32)
    b_uiou = np.asarray(inputs["b_uiou"], np.float32)

    Wio = W_iou * ln_w[None, :]
    c3 = (W_iou @ ln_b + b_wiou + b_uiou).astype(np.float32)   # [3H]
    # device iou chunk order [i0, o0, i1, o1, u0, u1] (chunks of 128)
    chunk_order = [0, 2, 1, 3, 4, 5]   # i0,o0,i1,o1,u0,u1 from [i0,i1,o0,o1,u0,u1]
    Wio_r = Wio.reshape(6, 128, H)[chunk_order]         # [6,128,H]
    c3_r = c3.reshape(6, 128)[chunk_order]              # [6,128]

    shared = {
        "w1d": np.ascontiguousarray(
            (W1.T * 16.0).reshape(KH * 3, 128, H).transpose(1, 0, 2)
        ).astype(f8),
        "b1d": np.ascontiguousarray((b1 * 256.0).reshape(KH, 128).T),
        "w2d": np.ascontiguousarray(
            W2.T.reshape(KH, 128, H).transpose(1, 0, 2)).astype(bf16),
        "b2d": np.ascontiguousarray(b2.reshape(KH, 128).T),
        "w3d": np.ascontiguousarray(
            (Wio_r.transpose(2, 0, 1) * 16.0)       # [H, 6, 128]
            .reshape(KH, 128, 6 * 128).transpose(1, 0, 2)
        ).astype(f8),
        "c3d": np.ascontiguousarray(c3_r.T),        # [128, 6]
    }
    in_maps = []
    for c in range(ncores):
        xs = x[c * npc:(c + 1) * npc]
        xtt = ((xs * 16.0).reshape(ntiles, nt, D // 128, 128)
               .transpose(0, 3, 2, 1)).astype(f8)
        in_maps.append({"xtt": np.ascontiguousarray(xtt), **shared})
    return in_maps


def host_finish(results, inputs, ncores):
    """Sum per-core pooled partials, apply the classifier on host (fp32)."""
    acc = np.zeros((128, KH_G := H // 128), np.float64)
    for c in range(ncores):
        acc += np.asarray(results[c]["out"], np.float64)
    pooled = acc.T.reshape(1, H).astype(np.float32) / float(N_TOTAL)
    Wc1 = np.asarray(inputs["Wc1"], np.float32)
    bc1 = np.asarray(inputs["bc1"], np.float32)
    Wc2 = np.asarray(inputs["Wc2"], np.float32)
    bc2 = np.asarray(inputs["bc2"], np.float32)
    z = np.maximum(pooled @ Wc1.T + bc1, 0.0)
    return np.ascontiguousarray((z @ Wc2.T + bc2).astype(np.float32))


def kernel(**inputs):
    from concourse.bass_utils import run_bass_kernel_spmd

    npc = N_TOTAL // NCORES
    nt = 500
    key = (npc, nt, NCORES)
    if key not in _CACHE:
        _CACHE[key] = build_nc(npc, nt, NCORES)
    nc = _CACHE[key]
    in_maps = host_prep(inputs, npc, nt, NCORES)
    res = run_bass_kernel_spmd(nc, in_maps, core_ids=list(range(NCORES)))
    return host_finish(res.results, inputs, NCORES)
